# revision 1
# baseline (speedup 1.0000x reference)
"""Trainium2 Bass kernel for nn_ContrastiveLoss_66030827208766.

Strategy (data-parallel over images, captions replicated):
  - 8 cores, 16 images each.  Images are assigned to cores by GLOBAL
    length rank (core = rank % 8, slot = rank // 8), so every core's
    slot-k image has nearly the same valid-object count.  Only valid
    objects are shipped, padded per slot-group to a shared width: group
    A = slots 0-7 padded to Wa = len_sorted[63], group B = slots 8-15
    padded to Wb = len_sorted[127].  One program serves all cores.
  - Padding replicates object 0 (always valid), so a plain max over the
    padded block equals the masked max over valid objects.
  - All matmul operands are bf16 (PE accumulates fp32; end-to-end loss
    error ~1e-5).  Captions are replicated to every core in D-major
    layout [D, w*128 + c]: each 128-column slice is one caption word
    across all 128 captions.
  - Device per core: per caption word w, one matmul (stationary caption
    chunk [D,128], moving packed image-objects [D, C]) -> PSUM bank;
    grouped strided reduce_max over each slot's object block ->
    buf[c, w, slot]; reduce_sum over w; scale by 1/caption_len ->
    two [128 caps, 16 slots] tiles (parts sort by different keys) ->
    DRAM.
  - Host: unpermute slots of each part, add, then the (tiny) triplet
    margin loss reduction in numpy.

Codegen constraint: every TPB instruction can carry at most ONE sync
wait.  Three tactics keep us within it: (1) freshly-DMA'd tiles are
first touched by degenerate 1x1 "junk" matmuls so the real matmuls'
DMA-queue requirements are already observed by the PE; (2) buffers are
laid out so each writer hits a disjoint range (no spurious WAW chains);
(3) a post-pass strips waits that are redundant by construction
(same-engine in-order completion, per-queue DMA FIFO, barrier-covered
drain waits).
"""

import ml_dtypes
import numpy as np

import concourse.bass as bass
import concourse.mybir as mybir
from concourse import tile
from concourse.bass_utils import run_bass_kernel_spmd
from concourse.tile_rust import add_dep_helper

B = 128          # batch (images == captions)
O1, W1 = 36, 50  # part 1: im objects, s words
O2, W2 = 25, 30  # part 2: pred objects, c_r words
D = 128
NCORES = 8
IPC = B // NCORES  # images (slots) per core
G = IPC // 2       # slots per width-group
MARGIN = 0.2
F32 = mybir.dt.float32
BF16 = mybir.dt.bfloat16

LAST_RESULT = None   # BassKernelResults of the most recent run (for test.py)
_NC = None           # cached program
_NC_KEY = None       # widths the cached program was built for


def _build_part(nc, pending, hoist, cap, imt, buf, ps_tiles, ps_cols,
                W, Wa, Wb, cap_piece_cols, cap_key):
    """Emit matmul + grouped-reduce stream for one t2i part.

    Chunk layout: if C = 8*(Wa+Wb) fits one PSUM bank, chunk j of a tile
    sits in bank j (group A at +0, group B at +8*Wa); otherwise each
    chunk takes two banks (A at +0, B at +512).
    """
    C = G * (Wa + Wb)
    if C <= 512:
        banks_per_chunk, offA, offB = 1, 0, G * Wa
    else:
        banks_per_chunk, offA, offB = 2, 0, 512
    wc_per_piece = cap_piece_cols // B

    w = 0
    t_idx = 0
    while w < W:
        ps = ps_tiles[t_idx % len(ps_tiles)]
        cap_chunks = ps_cols[t_idx % len(ps_tiles)] // (512 * banks_per_chunk)
        n = min(cap_chunks, W - w)
        t_idx += 1
        for j in range(n):
            pc = (w + j) // wc_per_piece
            hoist((cap_key, pc),
                  cap[:1, pc * cap_piece_cols:pc * cap_piece_cols + 1])
            cs = cap[:, (w + j) * B:(w + j + 1) * B]
            base = j * banks_per_chunk * 512
            if banks_per_chunk == 1:
                mm = nc.tensor.matmul(ps[:, base:base + C], cs, imt[:],
                                      start=True, stop=True)
                while pending:
                    add_dep_helper(mm.ins, pending.pop().ins, sync=False,
                                   reason="order matmul after wait-carrier")
            else:
                for off, w0, wid in ((offA, 0, Wa), (offB, G * Wa, Wb)):
                    mm = nc.tensor.matmul(
                        ps[:, base + off:base + off + G * wid], cs,
                        imt[:, w0:w0 + G * wid], start=True, stop=True)
                    while pending:
                        add_dep_helper(mm.ins, pending.pop().ins, sync=False,
                                       reason="order matmul after wait-carrier")
        # Two grouped reduces (uniform width within each) covering all n
        # chunks of this tile.
        stride = banks_per_chunk * 512
        v = ps[:, :n * stride].rearrange("p (c x) -> p c x", c=n)
        for off, wid, s0, s1 in ((offA, Wa, 0, G), (offB, Wb, G, IPC)):
            nc.vector.reduce_max(
                buf[:, w:w + n, s0:s1],
                v[:, :, off:off + G * wid].rearrange(
                    "p c (g o) -> p c g o", o=wid),
                axis=mybir.AxisListType.X,
            )
        w += n


def _build_nc(widths):
    (Wa1, Wb1, Wa2, Wb2) = widths
    nc = bass.Bass()
    C1 = G * (Wa1 + Wb1)
    C2 = G * (Wa2 + Wb2)
    capT1 = nc.dram_tensor("capT1", [D, B * W1], BF16, kind="ExternalInput")
    capT2 = nc.dram_tensor("capT2", [D, B * W2], BF16, kind="ExternalInput")
    imT1 = nc.dram_tensor("imT1", [D, C1], BF16, kind="ExternalInput")
    imT2 = nc.dram_tensor("imT2", [D, C2], BF16, kind="ExternalInput")
    rblob = nc.dram_tensor("rblob", [B, 2], F32, kind="ExternalInput")
    out_t = nc.dram_tensor("scores_t", [B, 2 * IPC], F32,
                           kind="ExternalOutput")

    with tile.TileContext(nc) as tc:
        with (
            tc.tile_pool(name="const", bufs=1) as cpool,
            tc.tile_pool(name="psum", bufs=1, space="PSUM") as pspool,
            tc.tile_pool(name="work", bufs=1) as wpool,
        ):
            # ---- input DMAs: 16 total, alternating the two HWDGE rings
            # (even index -> sync, odd -> scalar).  DMAHW bookkeeping
            # lanes are assigned by global round-robin, so each of the 8
            # lanes sees a single issuing engine -> per-lane FIFO holds
            # and own-lane waits are strippable.  The output DMA is
            # emission #16 -> lane 0 (sync), same engine as lane 0's
            # inputs.
            dma_idx = [0]

            def load(dst_ap, src_ap):
                eng = nc.sync if dma_idx[0] % 2 == 0 else nc.scalar
                dma_idx[0] += 1
                return eng.dma_start(dst_ap, src_ap)

            NP1 = 10  # cap1 pieces (5 w-chunks each): early words land early
            NP2 = 1   # cap2: one DMA, only needed after part 1 finishes
            P1C = B * W1 // NP1
            P2C = B * W2 // NP2

            imt1 = cpool.tile([D, C1], BF16, tag="imt1")
            load(imt1[:], imT1[:])
            cap1 = cpool.tile([D, B * W1], BF16, tag="cap1")
            for j in range(NP1):
                load(cap1[:, j * P1C:(j + 1) * P1C],
                     capT1[:, j * P1C:(j + 1) * P1C])
            imt2 = cpool.tile([D, C2], BF16, tag="imt2")
            load(imt2[:], imT2[:])
            cap2 = cpool.tile([D, B * W2], BF16, tag="cap2")
            for j in range(NP2):
                load(cap2[:, j * P2C:(j + 1) * P2C],
                     capT2[:, j * P2C:(j + 1) * P2C])
            rblob_sb = cpool.tile([B, 2], F32, tag="rblob")
            load(rblob_sb[:], rblob[:])
            r1 = rblob_sb[:, 0:1]
            r2 = rblob_sb[:, 1:2]
            # 14 input DMAs; the output DMA is emission #14 -> lane 6,
            # whose earlier user (emission #6) is also sync-issued.
            assert dma_idx[0] == 14, dma_idx

            # w-major: each reduce writes a disjoint contiguous-ish range.
            buf1 = wpool.tile([B, W1, IPC], F32, tag="buf1")
            buf2 = wpool.tile([B, W2, IPC], F32, tag="buf2")

            # Static PSUM: 4-bank + 3-bank ping-pong tiles shared by both
            # parts, 1 junk bank.  (Pool slot rotation would bundle both
            # accessor engines' release waits onto one matmul.)
            psA = pspool.tile([B, 2048], F32, tag="psA", name="psA")
            psB = pspool.tile([B, 1536], F32, tag="psB", name="psB")
            junk_ps = pspool.tile([1, 1], F32, tag="junk_ps", name="junk_ps")

            hoisted = {}
            pending = []

            def hoist(key, corner_ap):
                if key in hoisted:
                    return
                hoisted[key] = nc.tensor.matmul(
                    junk_ps[:, :], corner_ap, corner_ap,
                    start=True, stop=True, skip_group_check=True,
                )
                pending.append(hoisted[key])

            sout = wpool.tile([B, 2 * IPC], F32, tag="sout")
            s1 = wpool.tile([B, IPC], F32, tag="s1")
            s2 = wpool.tile([B, IPC], F32, tag="s2")

            hoist(("imt1",), imt1[:1, :1])
            _build_part(nc, pending, hoist, cap1, imt1, buf1, [psA, psB],
                        [2048, 1536], W1, Wa1, Wb1, P1C, "cap1")
            # Part-1 epilogue emitted before part 2: the DVE executes its
            # queue in order, so this overlaps part-2 matmuls.
            nc.vector.reduce_sum(s1[:], buf1[:].rearrange("p w i -> p i w"),
                                 axis=mybir.AxisListType.X)
            nc.vector.tensor_scalar_mul(sout[:, :IPC], s1[:], r1)

            hoist(("imt2",), imt2[:1, :1])
            _build_part(nc, pending, hoist, cap2, imt2, buf2, [psA, psB],
                        [2048, 1536], W2, Wa2, Wb2, P2C, "cap2")
            nc.vector.reduce_sum(s2[:], buf2[:].rearrange("p w i -> p i w"),
                                 axis=mybir.AxisListType.X)
            nc.vector.tensor_scalar_mul(sout[:, IPC:], s2[:], r2)
            out_dma = nc.sync.dma_start(out_t[:], sout[:])

    # ---- wait-strip post-pass ----------------------------------------
    # Walrus codegen accepts at most one sync wait per instruction;
    # remove waits that are redundant by construction.
    out_q = {u.ant_name for u in out_dma.ins.sync_info.on_update
             if u.ant_name.startswith("DMAHW")}
    for bb in nc.main_func.blocks:
        for ins in bb.instructions:
            si = ins.sync_info
            if si is None:
                continue
            t = type(ins).__name__
            if t == "InstDrain" and len(si.on_wait) > 2:
                # Kernel-tail drain: engine completion is enforced by the
                # per-engine drains + EVSEM butterfly that follow, and
                # input-DMA completions are covered transitively by the
                # compute that consumed them.  Only the output DMA's
                # queue wait is load-bearing.
                drop = lambda w: w.ant_name not in out_q
            elif t == "InstMatmult":
                # WAW on a reused psum bank: the prior matmul's drain
                # (~128 cyc) finished >=2 matmul-streams earlier, so the
                # same-engine completion wait is dead.
                drop = lambda w: w.ant_name.startswith("PE_")
            elif getattr(ins, "engine", None) == mybir.EngineType.DVE:
                # DVE fully drains its pipe between ops; waits on earlier
                # DVE completions are satisfied at issue.
                drop = lambda w: w.ant_name.startswith("DVE_")
            elif t == "InstDMACopy":
                # Per-lane FIFO (single issuing engine per lane by
                # construction) makes own-lane waits redundant.
                own = {u.ant_name for u in si.on_update
                       if u.ant_name.startswith("DMAHW")}
                drop = lambda w: w.ant_name in own
            else:
                continue
            kept = [w for w in si.on_wait if not drop(w)]
            if len(kept) != len(si.on_wait):
                si.on_wait = kept
                ins.sync_info = si
    return nc


def _plan(lens, omax):
    """Global length-rank plan: order[r] = image of rank r; core r%8 slot
    r//8.  Group widths: Wa covers slots 0..G-1 (ranks < 64), Wb the
    rest."""
    lens = np.clip(np.asarray(lens, dtype=np.int64), 1, omax)
    order = np.argsort(lens, kind="stable")
    Wa = int(lens[order[NCORES * G - 1]])
    Wb = int(lens[order[B - 1]])
    return order, Wa, Wb


def _pack_images(x_bf, lens, order, Wa, Wb, core):
    """Build the packed, padded, D-major [D, G*(Wa+Wb)] bf16 image-object
    matrix for one core.  Slot k = image order[8k + core]; its first
    lens[i] objects, padded to the group width by replicating object 0."""
    cols = []
    for k in range(IPC):
        i = order[NCORES * k + core]
        wid = Wa if k < G else Wb
        L = min(int(lens[i]), wid)
        blk = np.empty((wid, D), dtype=x_bf.dtype)
        blk[:L] = x_bf[i, :L]
        blk[L:] = x_bf[i, 0]
        cols.append(blk)
    return np.ascontiguousarray(np.concatenate(cols, axis=0).T)


def kernel(im, im_l, s, s_l, pred, pred_l, cap_o_pred, cap_o_l, c_r_pred,
           c_r_l, trace=False, tmpdir=None):
    global LAST_RESULT, _NC, _NC_KEY
    im = np.asarray(im, dtype=np.float32)
    s = np.asarray(s, dtype=np.float32)
    pred = np.asarray(pred, dtype=np.float32)
    c_r_pred = np.asarray(c_r_pred, dtype=np.float32)
    im_l = np.asarray(im_l)
    pred_l = np.asarray(pred_l)

    order1, Wa1, Wb1 = _plan(im_l, O1)
    order2, Wa2, Wb2 = _plan(pred_l, O2)
    widths = (Wa1, Wb1, Wa2, Wb2)

    im_bf = im.astype(ml_dtypes.bfloat16)
    pred_bf = pred.astype(ml_dtypes.bfloat16)

    def dmajor16(x):
        b, w, d = x.shape
        t = np.ascontiguousarray(x.transpose(1, 0, 2).reshape(w * b, d).T)
        return t.astype(ml_dtypes.bfloat16)

    capT1 = dmajor16(s)
    capT2 = dmajor16(c_r_pred)
    rblob = np.stack([1.0 / np.asarray(s_l, dtype=np.float32),
                      1.0 / np.asarray(c_r_l, dtype=np.float32)], axis=1)

    in_maps = []
    for m in range(NCORES):
        in_maps.append({
            "capT1": capT1,
            "capT2": capT2,
            "imT1": _pack_images(im_bf, im_l, order1, Wa1, Wb1, m),
            "imT2": _pack_images(pred_bf, pred_l, order2, Wa2, Wb2, m),
            "rblob": rblob,
        })

    if _NC is None or _NC_KEY != widths:
        _NC = _build_nc(widths)
        _NC_KEY = widths
    res = run_bass_kernel_spmd(_NC, in_maps, list(range(NCORES)), trace=trace,
                               tmpdir=tmpdir)
    LAST_RESULT = res

    # Each core returns [128 caps, 32]: part-1 slots then part-2 slots,
    # already scaled by 1/caption_len.  Unpermute slots back to image
    # order and add the parts.
    scores = np.zeros((B, B), dtype=np.float32)
    for m in range(NCORES):
        tile_m = res.results[m]["scores_t"]  # [128, 32]
        idx1 = order1[np.arange(IPC) * NCORES + m]
        idx2 = order2[np.arange(IPC) * NCORES + m]
        scores[idx1, :] += tile_m[:, :IPC].T
        scores[idx2, :] += tile_m[:, IPC:].T

    # Triplet margin loss on the full (tiny) B x B matrix.
    d = np.diag(scores).copy()
    cost_s = np.maximum(MARGIN + scores - d[:, None], 0.0).astype(np.float32)
    cost_im = np.maximum(MARGIN + scores - d[None, :], 0.0).astype(np.float32)
    np.fill_diagonal(cost_s, 0.0)
    np.fill_diagonal(cost_im, 0.0)
    out = cost_s.max(axis=1).sum() + cost_im.max(axis=0).sum()
    return np.asarray(out, dtype=np.float32)



# revision 18
# speedup vs baseline: 1.1332x; 1.1332x over previous
"""Trainium2 Bass kernel for nn_ContrastiveLoss_66030827208766.

Strategy (data-parallel over images, captions replicated):
  - 8 cores, 16 images each, assigned by GLOBAL length rank (core = rank
    % 8, slot = rank // 8) so all cores share one program.  Only valid
    objects are shipped, padded per slot-group to a shared width:
    group A = slots 0-7 padded to Wa = len_sorted[63], group B = slots
    8-15 padded to Wb = len_sorted[127].  Padding replicates object 0.
  - All inputs are packed into ONE bf16 DRAM blob [128, TOT] per core
    (imt1|imt2|cap1|cap2) and streamed with a few large-row DMAs
    (>=3 KB per descriptor) alternating the two HWDGE rings, so the
    PE can start ~2 us after the preamble and never starves.
  - Per caption word w: one matmul (stationary cap chunk [D,128],
    moving packed image objects [D,C]) -> one PSUM bank (8-bank ring).
  - Max-pool over objects is split across two engines:
      * "staged" words: ACT copies the PSUM bank group to SBUF as bf16
        (1 elem/cycle @1.2 GHz); the DVE then runs pairwise tensor_max
        ladders on the bf16 stage (2x mode: ~0.6 ns/elem) finishing
        with a small grouped reduce, writing per-(cap,slot) maxes.
      * "direct" words (interleaved + stream tail): DVE grouped
        reduce_max straight from PSUM (1x, fp32).
    buf is stage-ordered (sum over words is order-invariant).
  - reduce_sum over words per part -> sout [128 caps, 32]; host divides
    by caption lengths, unpermutes slots, adds parts, and runs the
    (tiny) triplet margin loss in numpy.

Codegen constraint: every TPB instruction can carry at most ONE sync
wait.  Tactics: (1) freshly-DMA'd pieces are first touched by 1x1
"junk" matmuls so real matmuls inherit DMA-queue requirements by
engine order; (2) single-consumer-per-PSUM-bank-word; (3) a post-pass
strips waits that are redundant by construction.
"""

import ml_dtypes
import numpy as np

import concourse.bass as bass
import concourse.mybir as mybir
from concourse import tile
from concourse.bass_utils import run_bass_kernel_spmd
from concourse.tile_rust import add_dep_helper

B = 128          # batch (images == captions)
O1, W1 = 36, 50  # part 1: im objects, s words
O2, W2 = 25, 30  # part 2: pred objects, c_r words
D = 128
NCORES = 8
IPC = B // NCORES  # images (slots) per core
G = IPC // 2       # slots per width-group
MARGIN = 0.2
F32 = mybir.dt.float32
BF16 = mybir.dt.bfloat16

LAST_RESULT = None
_NC = None
_NC_KEY = None

# ---- schedule knobs -------------------------------------------------
# word i is DVE-direct if (i % 8) in MIX_DIRECT (ring-aligned: word
# index mod 8 == PSUM bank), or in the stream tail (last TAIL words).
MIX_DIRECT = (6, 7)
TAIL1, TAIL2 = 4, 4
NSB1, NSB2 = 2, 2     # ladder sub-blocks per part
COPY_SPAN = 3         # words per ACT copy
LADDER_STOP = 4       # ladder down to <= this width, then grouped reduce


def _plan_words(W, tail):
    """Return (staged_words, direct_words) index lists."""
    staged, direct = [], []
    for i in range(W):
        if i >= W - tail or (i % 8) in MIX_DIRECT:
            direct.append(i)
        else:
            staged.append(i)
    return staged, direct


def _ladder_schedule(w):
    """Halving steps from width w down to <= LADDER_STOP.
    Returns (steps, final_w): steps = list of (h, w) meaning
    x[0:h] = max(x[0:h], x[w-h:w]); new width = w - h."""
    steps = []
    while w > LADDER_STOP:
        h = w // 2
        steps.append((h, w))
        w = w - h
    return steps, w


def _build_nc(widths, plans, strip=True):
    (Wa1, Wb1, Wa2, Wb2) = widths
    C1 = G * (Wa1 + Wb1)
    C2 = G * (Wa2 + Wb2)
    assert C1 <= 512 and C2 <= 512
    # blob columns: imt1 | imt2 | cap1 | cap2
    OFF_I1, OFF_I2 = 0, C1
    OFF_C1 = C1 + C2
    OFF_C2 = OFF_C1 + B * W1
    TOT = OFF_C2 + B * W2

    nc = bass.Bass()
    blob_t = nc.dram_tensor("blob", [D, TOT], BF16, kind="ExternalInput")
    out_t = nc.dram_tensor("scores_t", [B, 2 * IPC], F32,
                           kind="ExternalOutput")

    with tile.TileContext(nc) as tc:
        with (
            tc.tile_pool(name="const", bufs=1) as cpool,
            tc.tile_pool(name="psum", bufs=1, space="PSUM") as pspool,
            tc.tile_pool(name="work", bufs=1) as wpool,
        ):
            sb = cpool.tile([D, TOT], BF16, tag="sb")

            # ---- input DMA pieces (word-aligned, large rows) -------
            # piece 0: imt1+imt2+cap1 words [0,8) -> needed first
            # then the rest of cap1 in 2 pieces, cap2 in 2 pieces.
            pieces = [
                (0, OFF_C1 + 8 * B),
                (OFF_C1 + 8 * B, OFF_C1 + 28 * B),
                (OFF_C1 + 28 * B, OFF_C2),
                (OFF_C2, OFF_C2 + 16 * B),
                (OFF_C2 + 16 * B, TOT),
            ]
            dma_engs = [nc.sync, nc.scalar]
            for j, (c0, c1) in enumerate(pieces):
                dma_engs[j % 2].dma_start(sb[:, c0:c1], blob_t[:, c0:c1])
            # word -> piece index (for junk hoisting)
            def piece_of(col):
                for j, (c0, c1) in enumerate(pieces):
                    if c0 <= col < c1:
                        return j
                raise AssertionError(col)

            # stage + output buffers
            (stg1_words, dir1_words), (stg2_words, dir2_words) = plans
            S1, S2 = len(stg1_words), len(stg2_words)
            stage1 = wpool.tile([B, S1 * C1], BF16, tag="stage1")
            stage2 = wpool.tile([B, S2 * C2], BF16, tag="stage2")
            buf1 = wpool.tile([B, IPC * W1], BF16, tag="buf1")
            buf2 = wpool.tile([B, IPC * W2], BF16, tag="buf2")
            sout = wpool.tile([B, 2 * IPC], F32, tag="sout")

            ps = pspool.tile([B, 4096], F32, tag="ps", name="ps")

            hoisted = {}
            pending = []

            def hoist(key, corner_col):
                if key in hoisted:
                    return
                hoisted[key] = nc.tensor.matmul(
                    ps[0:1, C1:C1 + 1], sb[0:1, corner_col:corner_col + 1],
                    sb[0:1, corner_col:corner_col + 1],
                    start=True, stop=True, skip_group_check=True,
                )
                pending.append(hoisted[key])

            ring = [0]  # global ring counter across both parts

            def emit_part(part, W, C, Wa, Wb, off_cap, off_im, stg_words,
                          dir_words, stage, buf, nsb):
                """Emit matmul stream + consumers for one t2i part."""
                stg_pos = {w: k for k, w in enumerate(stg_words)}
                dir_pos = {w: k for k, w in enumerate(dir_words)}
                S = len(stg_words)
                # sub-block boundaries in stage order
                sb_bounds = [round(S * i / nsb) for i in range(nsb + 1)]
                # pending ACT copy run (stage-contiguous words share a copy)
                copy_run = []   # list of (word, bank)
                dir_run = []    # list of (word, bank)

                def flush_copy():
                    if not copy_run:
                        return
                    k0 = stg_pos[copy_run[0][0]]
                    n = len(copy_run)
                    b0 = copy_run[0][1]
                    # banks are consecutive (never wrap: flushed at bank 7)
                    src = ps[:, b0 * 512:(b0 + n) * 512].rearrange(
                        "p (c x) -> p c x", c=n)[:, :, :C]
                    dst = stage[:, k0 * C:(k0 + n) * C].rearrange(
                        "p (c x) -> p c x", c=n)
                    nc.scalar.copy(dst, src)
                    copy_run.clear()

                def flush_direct():
                    if not dir_run:
                        return
                    k0 = dir_pos[dir_run[0][0]]
                    n = len(dir_run)
                    b0 = dir_run[0][1]
                    v = ps[:, b0 * 512:(b0 + n) * 512].rearrange(
                        "p (c x) -> p c x", c=n)
                    for off, wid, s0, s1 in ((0, Wa, 0, G),
                                             (G * Wa, Wb, G, IPC)):
                        # dst: buf[p, slot, word] word-cols = S + dir idx
                        dst = buf[:, :].rearrange(
                            "p (s w) -> p s w", s=IPC)[
                                :, s0:s1, S + k0:S + k0 + n].rearrange(
                                    "p s w -> p w s")
                        nc.vector.reduce_max(
                            dst,
                            v[:, :, off:off + G * wid].rearrange(
                                "p c (g o) -> p c g o", o=wid),
                            axis=mybir.AxisListType.X,
                        )
                    dir_run.clear()

                def emit_ladders(lo, hi):
                    """Pool stage rows [lo,hi) (stage order) into buf."""
                    if hi <= lo:
                        return
                    nw = hi - lo
                    for g0, wid, s0 in ((0, Wa, 0), (G * Wa, Wb, G)):
                        view = stage[:, lo * C:hi * C].rearrange(
                            "p (c x) -> p c x", c=nw)[
                                :, :, g0:g0 + G * wid].rearrange(
                                    "p c (g o) -> p c g o", o=wid)
                        steps, wfin = _ladder_schedule(wid)
                        for (h, wcur) in steps:
                            nc.vector.tensor_max(
                                view[:, :, :, 0:h], view[:, :, :, 0:h],
                                view[:, :, :, wcur - h:wcur])
                        dst = buf[:, :].rearrange(
                            "p (s w) -> p s w", s=IPC)[
                                :, s0:s0 + G, lo:hi].rearrange(
                                    "p s w -> p w s")
                        nc.vector.reduce_max(
                            dst, view[:, :, :, 0:wfin],
                            axis=mybir.AxisListType.X)

                # align the ring so word index mod 8 == bank
                ring[0] = -(-ring[0] // 8) * 8
                next_sb = 1  # next sub-block boundary to emit
                for w in range(W):
                    bank = ring[0] % 8
                    ring[0] += 1
                    col = off_cap + w * B
                    hoist(piece_of(col), col)
                    if w == 0:
                        hoist(piece_of(off_im), off_im)
                    cs = sb[:, col:col + B]
                    imt = sb[:, off_im:off_im + C]
                    mm = nc.tensor.matmul(ps[:, bank * 512:bank * 512 + C],
                                          cs, imt, start=True, stop=True)
                    while pending:
                        add_dep_helper(mm.ins, pending.pop().ins, sync=False,
                                       reason="order after wait-carrier")
                    if w in stg_pos:
                        # direct runs cannot extend across a staged word
                        flush_direct()
                        copy_run.append((w, bank))
                        if (len(copy_run) == COPY_SPAN or w == W - 1
                                or bank == 7):
                            flush_copy()
                    else:
                        flush_copy()
                        dir_run.append((w, bank))
                        if len(dir_run) == 2 or w == W - 1 or bank == 7:
                            flush_direct()
                    # emit ladder sub-block once its stage rows are copied
                    done = stg_pos[w] + 1 if w in stg_pos else None
                    if (done is not None and next_sb <= nsb
                            and done >= sb_bounds[next_sb] and not copy_run):
                        emit_ladders(sb_bounds[next_sb - 1],
                                     sb_bounds[next_sb])
                        next_sb += 1
                flush_copy()
                flush_direct()
                while next_sb <= nsb:
                    emit_ladders(sb_bounds[next_sb - 1], sb_bounds[next_sb])
                    next_sb += 1
                # sum over words -> sout columns
                so = sout[:, (part - 1) * IPC:part * IPC]
                nc.vector.reduce_sum(
                    so, buf[:, :].rearrange("p (s w) -> p s w", s=IPC),
                    axis=mybir.AxisListType.X)

            emit_part(1, W1, C1, Wa1, Wb1, OFF_C1, OFF_I1,
                      stg1_words, dir1_words, stage1, buf1, NSB1)
            emit_part(2, W2, C2, Wa2, Wb2, OFF_C2, OFF_I2,
                      stg2_words, dir2_words, stage2, buf2, NSB2)
            out_dma = nc.sync.dma_start(out_t[:], sout[:])

    if not strip:
        return nc
    # ---- wait-strip post-pass ----------------------------------------
    # Cross-queue transitive closure.  Every sem here is updated from a
    # single queue (engine sems by their engine; each DMAHW lane by one
    # DMA in FIFO order), so "S >= k" pins a unique program point whose
    # guarantees (its queue's prior waits, recursively) we inherit.
    # A wait is dead if already guaranteed; own-queue sem waits are dead
    # by in-order completion.
    out_q = {u.ant_name for u in out_dma.ins.sync_info.on_update
             if u.ant_name.startswith("DMAHW")}
    eng_sem = {
        mybir.EngineType.PE: "PE_",
        mybir.EngineType.DVE: "DVE_",
        mybir.EngineType.Activation: "Activation_",
        mybir.EngineType.SP: "SP_",
        mybir.EngineType.Pool: "Pool_",
    }

    def covers(cov, name, val):
        return cov.get(name, -1) >= val

    def merge(cov, name, val):
        if cov.get(name, -1) < val:
            cov[name] = val

    # sems with any non-increment update or non-ge wait (barriers etc.)
    # are non-monotonic: exclude them from coverage and stripping.
    unsafe = set()
    for bb in nc.main_func.blocks:
        for ins in bb.instructions:
            si = ins.sync_info
            if si is None:
                continue
            for u in si.on_update:
                if getattr(u, "update_mode", "sem-inc") != "sem-inc":
                    unsafe.add(u.ant_name)
            for w in si.on_wait:
                if getattr(w, "wait_mode", "") != "sem-ge-imm":
                    unsafe.add(w.ant_name)

    semval = {}          # sem -> simulated value
    upd_cov = {}         # (sem, value) -> coverage snapshot at update
    qcov = {}            # queue -> cumulative guaranteed coverage
    for bb in nc.main_func.blocks:
        for ins in bb.instructions:
            si = ins.sync_info
            if si is None:
                continue
            t = type(ins).__name__
            eng = getattr(ins, "engine", None)
            own_pfx = eng_sem.get(eng, "\x00")
            cov = qcov.setdefault(eng, {})
            prior_cov = dict(cov)  # coverage BEFORE this instruction
            # fold this instruction's waits into future coverage (all of
            # them, stripped or not — stripping only removes redundancy)
            for w in si.on_wait:
                if getattr(w, "wait_mode", "") == "sem-ge-imm" \
                        and w.wait_value is not None \
                        and w.ant_name not in unsafe:
                    merge(cov, w.ant_name, w.wait_value)
                    prov = upd_cov.get((w.ant_name, w.wait_value))
                    if prov:
                        for n, v in prov.items():
                            merge(cov, n, v)
            if t == "InstDrain":
                # input-DMA completions are covered transitively by the
                # compute that consumed them; only the output DMA's
                # queue wait is load-bearing.
                kept = [w for w in si.on_wait
                        if not (w.ant_name.startswith("DMAHW")
                                and w.ant_name not in out_q)]
            elif t == "InstDMACopy":
                own = {u.ant_name for u in si.on_update
                       if u.ant_name.startswith("DMAHW")}
                kept = [w for w in si.on_wait
                        if w.ant_name not in own
                        and not w.ant_name.startswith(own_pfx)]
            else:
                kept = [w for w in si.on_wait
                        if not (w.ant_name.startswith(own_pfx)
                                and w.ant_name not in unsafe)]

            def implied(w):
                out = {w.ant_name: w.wait_value}
                for n, v in upd_cov.get(
                        (w.ant_name, w.wait_value), {}).items():
                    if out.get(n, -1) < v:
                        out[n] = v
                return out

            changed = True
            while changed and len(kept) > 1:
                changed = False
                for w in list(kept):
                    if getattr(w, "wait_mode", "") != "sem-ge-imm" \
                            or w.wait_value is None \
                            or w.ant_name in unsafe:
                        continue
                    avail = dict(prior_cov)
                    for w2 in kept:
                        if w2 is w:
                            continue
                        if getattr(w2, "wait_mode", "") == "sem-ge-imm" \
                                and w2.wait_value is not None \
                                and w2.ant_name not in unsafe:
                            for n, v in implied(w2).items():
                                merge(avail, n, v)
                    if covers(avail, w.ant_name, w.wait_value):
                        kept.remove(w)
                        changed = True
            # singleton still redundant vs prior coverage alone
            kept = [w for w in kept
                    if not (getattr(w, "wait_mode", "") == "sem-ge-imm"
                            and w.wait_value is not None
                            and w.ant_name not in unsafe
                            and covers(prior_cov, w.ant_name,
                                       w.wait_value))]
            if len(kept) != len(si.on_wait):
                si.on_wait = kept
                ins.sync_info = si
            # sem updates: snapshot coverage (includes this ins's waits)
            for u in si.on_update:
                name = u.ant_name
                if name in unsafe:
                    continue
                inc = getattr(u, "update_value", None)
                if inc is None:
                    inc = 16 if name.startswith("DMAHW") else 1
                semval[name] = semval.get(name, 0) + inc
                upd_cov[(name, semval[name])] = dict(cov)
    return nc


def _plan(lens, omax):
    lens = np.clip(np.asarray(lens, dtype=np.int64), 1, omax)
    order = np.argsort(lens, kind="stable")
    Wa = int(lens[order[NCORES * G - 1]])
    Wb = int(lens[order[B - 1]])
    return order, Wa, Wb


def _pack_images(x_bf, lens, order, Wa, Wb, core):
    cols = []
    for k in range(IPC):
        i = order[NCORES * k + core]
        wid = Wa if k < G else Wb
        L = min(int(lens[i]), wid)
        blk = np.empty((wid, D), dtype=x_bf.dtype)
        blk[:L] = x_bf[i, :L]
        blk[L:] = x_bf[i, 0]
        cols.append(blk)
    return np.concatenate(cols, axis=0).T  # [D, C]


def kernel(im, im_l, s, s_l, pred, pred_l, cap_o_pred, cap_o_l, c_r_pred,
           c_r_l, trace=False, tmpdir=None):
    global LAST_RESULT, _NC, _NC_KEY
    im = np.asarray(im, dtype=np.float32)
    s = np.asarray(s, dtype=np.float32)
    pred = np.asarray(pred, dtype=np.float32)
    c_r_pred = np.asarray(c_r_pred, dtype=np.float32)
    im_l = np.asarray(im_l)
    pred_l = np.asarray(pred_l)

    order1, Wa1, Wb1 = _plan(im_l, O1)
    order2, Wa2, Wb2 = _plan(pred_l, O2)
    widths = (Wa1, Wb1, Wa2, Wb2)
    plans = (_plan_words(W1, TAIL1), _plan_words(W2, TAIL2))

    im_bf = im.astype(ml_dtypes.bfloat16)
    pred_bf = pred.astype(ml_dtypes.bfloat16)

    def dmajor16(x):
        b, w, d = x.shape
        return x.transpose(1, 0, 2).reshape(w * b, d).T.astype(
            ml_dtypes.bfloat16)

    capT1 = dmajor16(s)          # [D, B*W1]
    capT2 = dmajor16(c_r_pred)   # [D, B*W2]

    in_maps = []
    for m in range(NCORES):
        imt1 = _pack_images(im_bf, im_l, order1, Wa1, Wb1, m)
        imt2 = _pack_images(pred_bf, pred_l, order2, Wa2, Wb2, m)
        blob = np.ascontiguousarray(
            np.concatenate([imt1, imt2, capT1, capT2], axis=1))
        in_maps.append({"blob": blob})

    key = (widths, tuple(tuple(x) for p in plans for x in p))
    if _NC is None or _NC_KEY != key:
        _NC = _build_nc(widths, plans)
        _NC_KEY = key
    res = run_bass_kernel_spmd(_NC, in_maps, list(range(NCORES)),
                               trace=trace, tmpdir=tmpdir)
    LAST_RESULT = res

    # ---- host epilogue ---------------------------------------------
    # Each core returns sout [128 caps, 32]: part-1 slots then part-2
    # slots, UNSCALED.  Scale by 1/caption_len, unpermute, add parts.
    inv1 = 1.0 / np.clip(np.asarray(s_l, dtype=np.float32), 1, None)
    inv2 = 1.0 / np.clip(np.asarray(c_r_l, dtype=np.float32), 1, None)
    scores = np.zeros((B, B), dtype=np.float32)
    for m in range(NCORES):
        tile_m = res.results[m]["scores_t"]  # [128, 32]
        idx1 = order1[np.arange(IPC) * NCORES + m]
        idx2 = order2[np.arange(IPC) * NCORES + m]
        scores[idx1, :] += (tile_m[:, :IPC] * inv1[:, None]).T
        scores[idx2, :] += (tile_m[:, IPC:] * inv2[:, None]).T

    d = np.diag(scores).copy()
    cost_s = np.maximum(MARGIN + scores - d[:, None], 0.0).astype(np.float32)
    cost_im = np.maximum(MARGIN + scores - d[None, :], 0.0).astype(np.float32)
    np.fill_diagonal(cost_s, 0.0)
    np.fill_diagonal(cost_im, 0.0)
    out = cost_s.max(axis=1).sum() + cost_im.max(axis=0).sum()
    return np.asarray(out, dtype=np.float32)


# revision 19
# speedup vs baseline: 1.1466x; 1.0118x over previous
"""Trainium2 Bass kernel for nn_ContrastiveLoss_66030827208766.

Strategy (data-parallel over images, captions replicated):
  - 8 cores, 16 images each, assigned by GLOBAL length rank (core = rank
    % 8, slot = rank // 8) so all cores share one program.  Only valid
    objects are shipped, padded per slot-group to a shared width:
    group A = slots 0-7 padded to Wa = len_sorted[63], group B = slots
    8-15 padded to Wb = len_sorted[127].  Padding replicates object 0.
  - All inputs are packed into ONE bf16 DRAM blob [128, TOT] per core
    (imt1|imt2|cap1|cap2) and streamed with a few large-row DMAs
    (>=3 KB per descriptor) alternating the two HWDGE rings, so the
    PE can start ~2 us after the preamble and never starves.
  - Per caption word w: one matmul (stationary cap chunk [D,128],
    moving packed image objects [D,C]) -> one PSUM bank (8-bank ring).
  - Max-pool over objects is split across two engines:
      * "staged" words: ACT copies the PSUM bank group to SBUF as bf16
        (1 elem/cycle @1.2 GHz); the DVE then runs pairwise tensor_max
        ladders on the bf16 stage (2x mode: ~0.6 ns/elem) finishing
        with a small grouped reduce, writing per-(cap,slot) maxes.
      * "direct" words (interleaved + stream tail): DVE grouped
        reduce_max straight from PSUM (1x, fp32).
    buf is stage-ordered (sum over words is order-invariant).
  - reduce_sum over words per part -> sout [128 caps, 32]; host divides
    by caption lengths, unpermutes slots, adds parts, and runs the
    (tiny) triplet margin loss in numpy.

Codegen constraint: every TPB instruction can carry at most ONE sync
wait.  Tactics: (1) freshly-DMA'd pieces are first touched by 1x1
"junk" matmuls so real matmuls inherit DMA-queue requirements by
engine order; (2) single-consumer-per-PSUM-bank-word; (3) a post-pass
strips waits that are redundant by construction.
"""

import ml_dtypes
import numpy as np

import concourse.bass as bass
import concourse.mybir as mybir
from concourse import tile
from concourse.bass_utils import run_bass_kernel_spmd
from concourse.tile_rust import add_dep_helper

B = 128          # batch (images == captions)
O1, W1 = 36, 50  # part 1: im objects, s words
O2, W2 = 25, 30  # part 2: pred objects, c_r words
D = 128
NCORES = 8
IPC = B // NCORES  # images (slots) per core
G = IPC // 2       # slots per width-group
MARGIN = 0.2
F32 = mybir.dt.float32
BF16 = mybir.dt.bfloat16

LAST_RESULT = None
_NC = None
_NC_KEY = None

# ---- schedule knobs -------------------------------------------------
# word i is DVE-direct if (i % 8) in MIX_DIRECT (ring-aligned: word
# index mod 8 == PSUM bank), or in the stream tail (last TAIL words).
MIX_DIRECT = (6, 7)
TAIL1, TAIL2 = 4, 4
NSB1, NSB2 = 2, 2     # ladder sub-blocks per part
COPY_SPAN = 3         # words per ACT copy
LADDER_STOP = 4       # ladder down to <= this width, then grouped reduce


def _plan_words(W, tail):
    """Return (staged_words, direct_words) index lists."""
    staged, direct = [], []
    for i in range(W):
        if i >= W - tail or (i % 8) in MIX_DIRECT:
            direct.append(i)
        else:
            staged.append(i)
    return staged, direct


def _ladder_schedule(w):
    """Halving steps from width w down to <= LADDER_STOP.
    Returns (steps, final_w): steps = list of (h, w) meaning
    x[0:h] = max(x[0:h], x[w-h:w]); new width = w - h.
    h and all offsets stay EVEN so the DVE 2x uop mode applies
    (odd counts/offsets fall back to 1x)."""
    assert w % 2 == 0, w
    steps = []
    while w > LADDER_STOP:
        h = (w // 2) & ~1
        steps.append((h, w))
        w = w - h
    return steps, w


def _build_nc(widths, plans, strip=True):
    (Wa1, Wb1, Wa2, Wb2) = widths
    C1 = G * (Wa1 + Wb1)
    C2 = G * (Wa2 + Wb2)
    assert C1 <= 512 and C2 <= 512
    # blob columns: imt1 | imt2 | cap1 | cap2
    OFF_I1, OFF_I2 = 0, C1
    OFF_C1 = C1 + C2
    OFF_C2 = OFF_C1 + B * W1
    TOT = OFF_C2 + B * W2

    nc = bass.Bass()
    blob_t = nc.dram_tensor("blob", [D, TOT], BF16, kind="ExternalInput")
    out_t = nc.dram_tensor("scores_t", [B, 2 * IPC], F32,
                           kind="ExternalOutput")

    with tile.TileContext(nc) as tc:
        with (
            tc.tile_pool(name="const", bufs=1) as cpool,
            tc.tile_pool(name="psum", bufs=1, space="PSUM") as pspool,
            tc.tile_pool(name="work", bufs=1) as wpool,
        ):
            sb = cpool.tile([D, TOT], BF16, tag="sb")

            # ---- input DMA pieces (word-aligned, large rows) -------
            # piece 0: imt1+imt2+cap1 words [0,8) -> needed first
            # then the rest of cap1 in 2 pieces, cap2 in 2 pieces.
            pieces = [
                (0, OFF_C1 + 2 * B),
                (OFF_C1 + 2 * B, OFF_C1 + 18 * B),
                (OFF_C1 + 18 * B, OFF_C1 + 34 * B),
                (OFF_C1 + 34 * B, OFF_C2 + 4 * B),
                (OFF_C2 + 4 * B, OFF_C2 + 18 * B),
                (OFF_C2 + 18 * B, TOT),
            ]
            dma_engs = [nc.sync, nc.scalar]
            for j, (c0, c1) in enumerate(pieces):
                dma_engs[j % 2].dma_start(sb[:, c0:c1], blob_t[:, c0:c1])
            # word -> piece index (for junk hoisting)
            def piece_of(col):
                for j, (c0, c1) in enumerate(pieces):
                    if c0 <= col < c1:
                        return j
                raise AssertionError(col)

            # stage + output buffers
            (stg1_words, dir1_words), (stg2_words, dir2_words) = plans
            S1, S2 = len(stg1_words), len(stg2_words)
            stage1 = wpool.tile([B, S1 * C1], BF16, tag="stage1")
            stage2 = wpool.tile([B, S2 * C2], BF16, tag="stage2")
            buf1 = wpool.tile([B, IPC * W1], BF16, tag="buf1")
            buf2 = wpool.tile([B, IPC * W2], BF16, tag="buf2")
            sout = wpool.tile([B, 2 * IPC], F32, tag="sout")

            ps = pspool.tile([B, 4096], F32, tag="ps", name="ps")

            hoisted = {}
            pending = []

            def hoist(key, corner_col):
                if key in hoisted:
                    return
                hoisted[key] = nc.tensor.matmul(
                    ps[0:1, C1:C1 + 1], sb[0:1, corner_col:corner_col + 1],
                    sb[0:1, corner_col:corner_col + 1],
                    start=True, stop=True, skip_group_check=True,
                )
                pending.append(hoisted[key])

            ring = [0]  # global ring counter across both parts

            def emit_part(part, W, C, Wa, Wb, off_cap, off_im, stg_words,
                          dir_words, stage, buf, nsb):
                """Emit matmul stream + consumers for one t2i part."""
                stg_pos = {w: k for k, w in enumerate(stg_words)}
                dir_pos = {w: k for k, w in enumerate(dir_words)}
                S = len(stg_words)
                # sub-block boundaries in stage order
                sb_bounds = [round(S * i / nsb) for i in range(nsb + 1)]
                # pending ACT copy run (stage-contiguous words share a copy)
                copy_run = []   # list of (word, bank)
                dir_run = []    # list of (word, bank)

                def flush_copy():
                    if not copy_run:
                        return
                    k0 = stg_pos[copy_run[0][0]]
                    n = len(copy_run)
                    b0 = copy_run[0][1]
                    # banks are consecutive (never wrap: flushed at bank 7)
                    src = ps[:, b0 * 512:(b0 + n) * 512].rearrange(
                        "p (c x) -> p c x", c=n)[:, :, :C]
                    dst = stage[:, k0 * C:(k0 + n) * C].rearrange(
                        "p (c x) -> p c x", c=n)
                    nc.scalar.copy(dst, src)
                    copy_run.clear()

                def flush_direct():
                    if not dir_run:
                        return
                    k0 = dir_pos[dir_run[0][0]]
                    n = len(dir_run)
                    b0 = dir_run[0][1]
                    v = ps[:, b0 * 512:(b0 + n) * 512].rearrange(
                        "p (c x) -> p c x", c=n)
                    for off, wid, s0, s1 in ((0, Wa, 0, G),
                                             (G * Wa, Wb, G, IPC)):
                        # dst: buf[p, slot, word] word-cols = S + dir idx
                        dst = buf[:, :].rearrange(
                            "p (s w) -> p s w", s=IPC)[
                                :, s0:s1, S + k0:S + k0 + n].rearrange(
                                    "p s w -> p w s")
                        nc.vector.reduce_max(
                            dst,
                            v[:, :, off:off + G * wid].rearrange(
                                "p c (g o) -> p c g o", o=wid),
                            axis=mybir.AxisListType.X,
                        )
                    dir_run.clear()

                def emit_ladders(lo, hi):
                    """Pool stage rows [lo,hi) (stage order) into buf."""
                    if hi <= lo:
                        return
                    nw = hi - lo
                    for g0, wid, s0 in ((0, Wa, 0), (G * Wa, Wb, G)):
                        view = stage[:, lo * C:hi * C].rearrange(
                            "p (c x) -> p c x", c=nw)[
                                :, :, g0:g0 + G * wid].rearrange(
                                    "p c (g o) -> p c g o", o=wid)
                        steps, wfin = _ladder_schedule(wid)
                        for (h, wcur) in steps:
                            nc.vector.tensor_max(
                                view[:, :, :, 0:h], view[:, :, :, 0:h],
                                view[:, :, :, wcur - h:wcur])
                        dst = buf[:, :].rearrange(
                            "p (s w) -> p s w", s=IPC)[
                                :, s0:s0 + G, lo:hi].rearrange(
                                    "p s w -> p w s")
                        nc.vector.reduce_max(
                            dst, view[:, :, :, 0:wfin],
                            axis=mybir.AxisListType.X)

                # align the ring so word index mod 8 == bank
                ring[0] = -(-ring[0] // 8) * 8
                next_sb = 1  # next sub-block boundary to emit
                for w in range(W):
                    bank = ring[0] % 8
                    ring[0] += 1
                    col = off_cap + w * B
                    hoist(piece_of(col), col)
                    if w == 0:
                        hoist(piece_of(off_im), off_im)
                    cs = sb[:, col:col + B]
                    imt = sb[:, off_im:off_im + C]
                    mm = nc.tensor.matmul(ps[:, bank * 512:bank * 512 + C],
                                          cs, imt, start=True, stop=True)
                    while pending:
                        add_dep_helper(mm.ins, pending.pop().ins, sync=False,
                                       reason="order after wait-carrier")
                    if w in stg_pos:
                        # direct runs cannot extend across a staged word
                        flush_direct()
                        copy_run.append((w, bank))
                        if (len(copy_run) == COPY_SPAN or w == W - 1
                                or bank == 7):
                            flush_copy()
                    else:
                        flush_copy()
                        dir_run.append((w, bank))
                        if len(dir_run) == 2 or w == W - 1 or bank == 7:
                            flush_direct()
                    # emit ladder sub-block once its stage rows are copied
                    done = stg_pos[w] + 1 if w in stg_pos else None
                    if (done is not None and next_sb <= nsb
                            and done >= sb_bounds[next_sb] and not copy_run):
                        emit_ladders(sb_bounds[next_sb - 1],
                                     sb_bounds[next_sb])
                        next_sb += 1
                flush_copy()
                flush_direct()
                while next_sb <= nsb:
                    emit_ladders(sb_bounds[next_sb - 1], sb_bounds[next_sb])
                    next_sb += 1
                # sum over words -> sout columns
                so = sout[:, (part - 1) * IPC:part * IPC]
                nc.vector.reduce_sum(
                    so, buf[:, :].rearrange("p (s w) -> p s w", s=IPC),
                    axis=mybir.AxisListType.X)

            emit_part(1, W1, C1, Wa1, Wb1, OFF_C1, OFF_I1,
                      stg1_words, dir1_words, stage1, buf1, NSB1)
            emit_part(2, W2, C2, Wa2, Wb2, OFF_C2, OFF_I2,
                      stg2_words, dir2_words, stage2, buf2, NSB2)
            out_dma = nc.sync.dma_start(out_t[:], sout[:])

    if not strip:
        return nc
    # ---- wait-strip post-pass ----------------------------------------
    # Cross-queue transitive closure.  Every sem here is updated from a
    # single queue (engine sems by their engine; each DMAHW lane by one
    # DMA in FIFO order), so "S >= k" pins a unique program point whose
    # guarantees (its queue's prior waits, recursively) we inherit.
    # A wait is dead if already guaranteed; own-queue sem waits are dead
    # by in-order completion.
    out_q = {u.ant_name for u in out_dma.ins.sync_info.on_update
             if u.ant_name.startswith("DMAHW")}
    eng_sem = {
        mybir.EngineType.PE: "PE_",
        mybir.EngineType.DVE: "DVE_",
        mybir.EngineType.Activation: "Activation_",
        mybir.EngineType.SP: "SP_",
        mybir.EngineType.Pool: "Pool_",
    }

    def covers(cov, name, val):
        return cov.get(name, -1) >= val

    def merge(cov, name, val):
        if cov.get(name, -1) < val:
            cov[name] = val

    # sems with any non-increment update or non-ge wait (barriers etc.)
    # are non-monotonic: exclude them from coverage and stripping.
    unsafe = set()
    for bb in nc.main_func.blocks:
        for ins in bb.instructions:
            si = ins.sync_info
            if si is None:
                continue
            for u in si.on_update:
                if getattr(u, "update_mode", "sem-inc") != "sem-inc":
                    unsafe.add(u.ant_name)
            for w in si.on_wait:
                if getattr(w, "wait_mode", "") != "sem-ge-imm":
                    unsafe.add(w.ant_name)

    semval = {}          # sem -> simulated value
    upd_cov = {}         # (sem, value) -> coverage snapshot at update
    qcov = {}            # queue -> cumulative guaranteed coverage
    for bb in nc.main_func.blocks:
        for ins in bb.instructions:
            si = ins.sync_info
            if si is None:
                continue
            t = type(ins).__name__
            eng = getattr(ins, "engine", None)
            own_pfx = eng_sem.get(eng, "\x00")
            cov = qcov.setdefault(eng, {})
            prior_cov = dict(cov)  # coverage BEFORE this instruction
            # fold this instruction's waits into future coverage (all of
            # them, stripped or not — stripping only removes redundancy)
            for w in si.on_wait:
                if getattr(w, "wait_mode", "") == "sem-ge-imm" \
                        and w.wait_value is not None \
                        and w.ant_name not in unsafe:
                    merge(cov, w.ant_name, w.wait_value)
                    prov = upd_cov.get((w.ant_name, w.wait_value))
                    if prov:
                        for n, v in prov.items():
                            merge(cov, n, v)
            if t == "InstDrain":
                # input-DMA completions are covered transitively by the
                # compute that consumed them; only the output DMA's
                # queue wait is load-bearing.
                kept = [w for w in si.on_wait
                        if not (w.ant_name.startswith("DMAHW")
                                and w.ant_name not in out_q)]
            elif t == "InstDMACopy":
                own = {u.ant_name for u in si.on_update
                       if u.ant_name.startswith("DMAHW")}
                kept = [w for w in si.on_wait
                        if w.ant_name not in own
                        and not w.ant_name.startswith(own_pfx)]
            else:
                kept = [w for w in si.on_wait
                        if not (w.ant_name.startswith(own_pfx)
                                and w.ant_name not in unsafe)]

            def implied(w):
                out = {w.ant_name: w.wait_value}
                for n, v in upd_cov.get(
                        (w.ant_name, w.wait_value), {}).items():
                    if out.get(n, -1) < v:
                        out[n] = v
                return out

            changed = True
            while changed and len(kept) > 1:
                changed = False
                for w in list(kept):
                    if getattr(w, "wait_mode", "") != "sem-ge-imm" \
                            or w.wait_value is None \
                            or w.ant_name in unsafe:
                        continue
                    avail = dict(prior_cov)
                    for w2 in kept:
                        if w2 is w:
                            continue
                        if getattr(w2, "wait_mode", "") == "sem-ge-imm" \
                                and w2.wait_value is not None \
                                and w2.ant_name not in unsafe:
                            for n, v in implied(w2).items():
                                merge(avail, n, v)
                    if covers(avail, w.ant_name, w.wait_value):
                        kept.remove(w)
                        changed = True
            # singleton still redundant vs prior coverage alone
            kept = [w for w in kept
                    if not (getattr(w, "wait_mode", "") == "sem-ge-imm"
                            and w.wait_value is not None
                            and w.ant_name not in unsafe
                            and covers(prior_cov, w.ant_name,
                                       w.wait_value))]
            if len(kept) != len(si.on_wait):
                si.on_wait = kept
                ins.sync_info = si
            # sem updates: snapshot coverage (includes this ins's waits)
            for u in si.on_update:
                name = u.ant_name
                if name in unsafe:
                    continue
                inc = getattr(u, "update_value", None)
                if inc is None:
                    inc = 16 if name.startswith("DMAHW") else 1
                semval[name] = semval.get(name, 0) + inc
                upd_cov[(name, semval[name])] = dict(cov)
    return nc


def _plan(lens, omax):
    lens = np.clip(np.asarray(lens, dtype=np.int64), 1, omax)
    order = np.argsort(lens, kind="stable")
    Wa = int(lens[order[NCORES * G - 1]])
    Wb = int(lens[order[B - 1]])
    # round up to even: keeps every ladder step 2x-eligible on the DVE
    Wa += Wa % 2
    Wb += Wb % 2
    return order, Wa, Wb


def _pack_images(x_bf, lens, order, Wa, Wb, core):
    cols = []
    for k in range(IPC):
        i = order[NCORES * k + core]
        wid = Wa if k < G else Wb
        L = min(int(lens[i]), wid)
        blk = np.empty((wid, D), dtype=x_bf.dtype)
        blk[:L] = x_bf[i, :L]
        blk[L:] = x_bf[i, 0]
        cols.append(blk)
    return np.concatenate(cols, axis=0).T  # [D, C]


def kernel(im, im_l, s, s_l, pred, pred_l, cap_o_pred, cap_o_l, c_r_pred,
           c_r_l, trace=False, tmpdir=None):
    global LAST_RESULT, _NC, _NC_KEY
    im = np.asarray(im, dtype=np.float32)
    s = np.asarray(s, dtype=np.float32)
    pred = np.asarray(pred, dtype=np.float32)
    c_r_pred = np.asarray(c_r_pred, dtype=np.float32)
    im_l = np.asarray(im_l)
    pred_l = np.asarray(pred_l)

    order1, Wa1, Wb1 = _plan(im_l, O1)
    order2, Wa2, Wb2 = _plan(pred_l, O2)
    widths = (Wa1, Wb1, Wa2, Wb2)
    plans = (_plan_words(W1, TAIL1), _plan_words(W2, TAIL2))

    im_bf = im.astype(ml_dtypes.bfloat16)
    pred_bf = pred.astype(ml_dtypes.bfloat16)

    def dmajor16(x):
        b, w, d = x.shape
        return x.transpose(1, 0, 2).reshape(w * b, d).T.astype(
            ml_dtypes.bfloat16)

    capT1 = dmajor16(s)          # [D, B*W1]
    capT2 = dmajor16(c_r_pred)   # [D, B*W2]

    in_maps = []
    for m in range(NCORES):
        imt1 = _pack_images(im_bf, im_l, order1, Wa1, Wb1, m)
        imt2 = _pack_images(pred_bf, pred_l, order2, Wa2, Wb2, m)
        blob = np.ascontiguousarray(
            np.concatenate([imt1, imt2, capT1, capT2], axis=1))
        in_maps.append({"blob": blob})

    key = (widths, tuple(tuple(x) for p in plans for x in p))
    if _NC is None or _NC_KEY != key:
        _NC = _build_nc(widths, plans)
        _NC_KEY = key
    res = run_bass_kernel_spmd(_NC, in_maps, list(range(NCORES)),
                               trace=trace, tmpdir=tmpdir)
    LAST_RESULT = res

    # ---- host epilogue ---------------------------------------------
    # Each core returns sout [128 caps, 32]: part-1 slots then part-2
    # slots, UNSCALED.  Scale by 1/caption_len, unpermute, add parts.
    inv1 = 1.0 / np.clip(np.asarray(s_l, dtype=np.float32), 1, None)
    inv2 = 1.0 / np.clip(np.asarray(c_r_l, dtype=np.float32), 1, None)
    scores = np.zeros((B, B), dtype=np.float32)
    for m in range(NCORES):
        tile_m = res.results[m]["scores_t"]  # [128, 32]
        idx1 = order1[np.arange(IPC) * NCORES + m]
        idx2 = order2[np.arange(IPC) * NCORES + m]
        scores[idx1, :] += (tile_m[:, :IPC] * inv1[:, None]).T
        scores[idx2, :] += (tile_m[:, IPC:] * inv2[:, None]).T

    d = np.diag(scores).copy()
    cost_s = np.maximum(MARGIN + scores - d[:, None], 0.0).astype(np.float32)
    cost_im = np.maximum(MARGIN + scores - d[None, :], 0.0).astype(np.float32)
    np.fill_diagonal(cost_s, 0.0)
    np.fill_diagonal(cost_im, 0.0)
    out = cost_s.max(axis=1).sum() + cost_im.max(axis=0).sum()
    return np.asarray(out, dtype=np.float32)
